# revision 32
# baseline (speedup 1.0000x reference)
"""MixLinear int4-GEMM kernel for 8x TRN2 NeuronCores.

Strategy: 2-way M x 4-way OUT sharding.  Core c = mg*4 + og owns rows
[mg*4096, (mg+1)*4096) and output channels [og*1024, (og+1)*1024).  This
splits the per-row quantization work 4x vs pure OUT-sharding (which
duplicated it on all 8 cores) while keeping the whole bf16 weight shard
resident in SBUF.

Per core:
  Setup (once): int4 weight shard unpacked on DVE into bf16 and DMA-xbar
  transposed to contraction-major wT; outlier weight columns wc/sc
  (pre-divided by scale_col so one dequant covers everything) transposed
  via PE into wcT bf16.
  Per 128-row tile (32 tiles): masked abs-max -> x_scale; magic-number
  RNE round on ScalarE+DVE -> q (exact ints in bf16); DMA-xbar transpose
  to qT; GPSIMD outlier gather + ScalarE scale + PE transpose; 32+2 bf16
  matmuls into a [128, 1024] PSUM pair; dequant eviction; DMA out.

KERNEL_SAFE env (comma list) falls back to baseline-proven constructs:
  mask  - f32 mask, masked quantize (TT+reduce, no fused TTR / weight-col
          zeroing / in-place DVE)
  evict - two-step dequant (ScalarE scale, then DVE col-scale)
  qtp1  - single-buffered qT
"""

import os as _os

import numpy as np

B, S, IN, OUT, FP = 4, 2048, 4096, 4096, 256
M = B * S
NCORES = 8
MG, OG = 2, 4            # M-groups x OUT-groups
MS = M // MG             # rows per core (4096)
OS = OUT // OG           # out-features per core (1024)
QMAX = 7.0
MAGIC = 12582912.0       # 1.5 * 2**23: add+subtract forces RNE to integer


def emit_core_kernel(nc, tc, m, in_dim, os_dim, fp_dim):
    """Emit the per-core tile program. All dims compile-time constants."""
    import concourse.mybir as mybir
    from concourse import library_config
    from concourse.masks import make_identity

    SAFE = set(_os.environ.get("KERNEL_SAFE", "").split(","))

    f32 = mybir.dt.float32
    f8 = mybir.dt.float8e4
    bf16 = mybir.dt.bfloat16
    i32 = mybir.dt.int32
    i16 = mybir.dt.int16
    Alu = mybir.AluOpType
    Act = mybir.ActivationFunctionType

    P = 128
    MT = m // P              # 32 activation row tiles
    KT = in_dim // P         # 32 contraction tiles
    FT = fp_dim // P         # 2 outlier contraction tiles
    OC = os_dim // P         # 8 out-shard 128-chunks
    NJ = os_dim // 512       # 2 psum chunks of 512

    x = nc.dram_tensor("x", [m, in_dim], f32, kind="ExternalInput")
    qw = nc.dram_tensor("qw", [os_dim, in_dim // 2], i32, kind="ExternalInput")
    sc = nc.dram_tensor("sc", [os_dim], f32, kind="ExternalInput")
    wc = nc.dram_tensor("wc", [os_dim, fp_dim], f32, kind="ExternalInput")
    maskrow = nc.dram_tensor("maskrow", [in_dim], f32, kind="ExternalInput")
    idx = nc.dram_tensor("idx", [P, fp_dim // 16], i16, kind="ExternalInput")
    y = nc.dram_tensor("y", [m, os_dim], f32, kind="ExternalOutput")

    with (
        tc.tile_pool(name="const", bufs=1) as const,
        tc.tile_pool(name="wstage", bufs=1) as wstage,
        tc.tile_pool(name="xp", bufs=2) as xp,
        tc.tile_pool(name="xzp", bufs=1) as xzp,
        tc.tile_pool(name="qp", bufs=2) as qp,
        tc.tile_pool(name="qtp", bufs=1 if "qtp1" in SAFE else 2) as qtp,
        tc.tile_pool(name="qfp", bufs=2) as qfp,
        tc.tile_pool(name="aop", bufs=2) as aop,
        tc.tile_pool(name="sp", bufs=8) as sp,
        tc.tile_pool(name="yp", bufs=1 if "evict" in SAFE else 2) as yp,
        tc.tile_pool(name="py", bufs=3, space="PSUM") as py,
        tc.tile_pool(name="ptp", bufs=2, space="PSUM") as ptp,
    ):
        # ---------------- one-time setup ----------------
        nc.gpsimd.load_library(library_config.ap_gather)

        identity = const.tile([P, P], f32)
        make_identity(nc, identity[:])

        # outlier mask broadcast to all partitions, bf16 (0/1 exact;
        # halves the mask read bandwidth in the masking TT pass)
        maskF = const.tile([P, in_dim], bf16, name="maskF")
        mtmp = xzp.tile([P, in_dim], f32, tag="xz")
        nc.sync.dma_start(mtmp[:], maskrow[None, :].to_broadcast((P, in_dim)))
        nc.scalar.activation(maskF[:], mtmp[:], Act.Copy)

        # wrapped gather indices for ap_gather
        idxs = const.tile([P, fp_dim // 16], i16)
        nc.sync.dma_start(idxs[:], idx[:])

        # scale_col shard broadcast along partitions [P, OS] for dequant
        sc_bcast = const.tile([P, os_dim], f32)
        nc.sync.dma_start(sc_bcast[:], sc[None, :].to_broadcast((P, os_dim)))

        # scale_col per-partition view [P, OC] for pre-dividing weight_cache
        sc_op = const.tile([P, OC], f32)
        nc.sync.dma_start(sc_op[:], sc.rearrange("(c p) -> p c", p=P))
        rsc_op = const.tile([P, OC], f32)
        nc.vector.reciprocal(rsc_op[:], sc_op[:])

        # int4 weight unpack into contraction-major bf16 wT (per j-half so
        # early matmuls only wait on their own half), and weight_cache
        # pre-scaled by 1/scale_col into wcT.
        wT = [
            const.tile([P, KT, 512], f8, name=f"wT{j}", tag=f"wT{j}")
            for j in range(NJ)
        ]
        wcT = const.tile([P, FT, os_dim], bf16)
        qw_v = qw.rearrange("(c p) k -> p c k", p=P)
        wc_v = wc.rearrange("(c p) f -> p c f", p=P)
        for c in range(OC):
            qwj = wstage.tile([P, in_dim // 2], i32, tag="qwj", bufs=2)
            nc.sync.dma_start(qwj[:], qw_v[:, c, :])
            w_ok = wstage.tile([P, in_dim], bf16, tag="wok")
            w_ok_v = w_ok.rearrange("p (k two) -> p k two", two=2)
            tmp = wstage.tile([P, in_dim // 2], i32, tag="wtmp")
            # low nibble: ((v & 15) ^ 8) - 8
            nc.vector.tensor_scalar(
                tmp[:], qwj[:], 15, 8, Alu.bitwise_and, Alu.bitwise_xor
            )
            nc.vector.tensor_scalar(w_ok_v[:, :, 0], tmp[:], 8, None, Alu.subtract)
            # high nibble: same decode after v >>= 4 (ping-pong, no in-place)
            nc.vector.tensor_scalar(tmp[:], qwj[:], 4, None, Alu.arith_shift_right)
            nc.vector.tensor_scalar(
                qwj[:], tmp[:], 15, 8, Alu.bitwise_and, Alu.bitwise_xor
            )
            nc.vector.tensor_scalar(w_ok_v[:, :, 1], qwj[:], 8, None, Alu.subtract)
            # transpose [128 o, in_dim k] -> staging, then convert to fp8
            # (exact: int4 values; DMA transpose only supports 2B dtypes)
            j, cc = c // (OC // NJ), c % (OC // NJ)
            wTs = wstage.tile([P, KT, P], bf16, tag="wts", bufs=2)
            nc.sync.dma_start_transpose(wTs[:], w_ok[:])
            nc.scalar.activation(
                wT[j][:, :, cc * P : (cc + 1) * P], wTs[:], Act.Copy
            )

            # outlier weights: wcp = wc[o, f] / sc[o], PE transpose (f32
            # PSUM; converted to bf16 at the ScalarE eviction)
            wcc = wstage.tile([P, fp_dim], f32, tag="wcc")
            nc.sync.dma_start(wcc[:], wc_v[:, c, :])
            wcp = wstage.tile([P, fp_dim], f32, tag="wcp")
            nc.scalar.activation(
                wcp[:], wcc[:], Act.Copy, scale=rsc_op[:, c : c + 1]
            )
            for ff in range(FT):
                ps = ptp.tile([P, P], f32, tag="tp")
                nc.tensor.transpose(
                    ps[:], wcp[:, ff * P : (ff + 1) * P], identity[:]
                )
                nc.scalar.activation(
                    wcT[:, ff, c * P : (c + 1) * P], ps[:], Act.Copy
                )

        # ---------------- main loop over 128-row activation tiles ----------
        inv7 = float(np.float32(1.0) / np.float32(QMAX))
        for mi in range(MT):
            x_t = xp.tile([P, in_dim], f32)
            nc.sync.dma_start(x_t[:], x[mi * P : (mi + 1) * P, :])

            # outlier activations (full precision, pre-masking)
            ao = aop.tile([P, fp_dim], f32, tag="ao")
            nc.gpsimd.ap_gather(
                ao[:, :, None],
                x_t[:, :, None],
                idxs[:],
                channels=P,
                num_elems=in_dim,
                d=1,
                num_idxs=fp_dim,
            )

            # xz = x*mask (one DVE pass, bf16 mask); mx = absmax(xz).
            # NOTE: the fused tensor_tensor_reduce hangs real TRN2 hardware
            # (passes CoreSim) -- keep the two-instruction form.
            axs = xzp.tile([P, in_dim], f32, tag="xz")
            mx = sp.tile([P, 1], f32, tag="mx")
            nc.vector.tensor_tensor(axs[:], x_t[:], maskF[:], Alu.mult)
            nc.vector.tensor_reduce(
                mx[:], axs[:], mybir.AxisListType.X, Alu.max,
                apply_absolute_value=True,
            )
            qsrc = axs
            s_t = sp.tile([P, 1], f32, tag="s")
            nc.vector.tensor_scalar(s_t[:], mx[:], inv7, None, Alu.mult)
            r_t = sp.tile([P, 1], f32, tag="r")
            nc.vector.reciprocal(r_t[:], s_t[:])

            # outliers scaled by r (ScalarE; per-partition scale), PE transpose
            aos = aop.tile([P, fp_dim], f32, tag="aos")
            nc.scalar.activation(aos[:], ao[:], Act.Copy, scale=r_t[:])
            aoT = aop.tile([P, FT, P], bf16, tag="aoT", bufs=1)
            for ff in range(FT):
                ps = ptp.tile([P, P], f32, tag="tp")
                nc.tensor.transpose(
                    ps[:], aos[:, ff * P : (ff + 1) * P], identity[:]
                )
                nc.scalar.activation(aoT[:, ff, :], ps[:], Act.Copy)

            # quantize: t = qsrc * r + MAGIC (ScalarE), q = t - MAGIC (DVE 2x).
            # t lands in whichever of x_t/axs is no longer needed.
            tq = x_t if qsrc is axs else axs
            nc.scalar.activation(tq[:], qsrc[:], Act.Copy, bias=MAGIC, scale=r_t[:])
            q = qp.tile([P, in_dim], bf16, tag="q")
            nc.scalar.activation(q[:], tq[:], Act.Copy, bias=-MAGIC)

            # transpose q to contraction-major via DMA xbar (bf16), then
            # DVE-convert to fp8 for the DoubleRow matmuls (exact ints)
            qTs = qtp.tile([P, KT, P], bf16)
            nc.sync.dma_start_transpose(qTs[:], q[:])
            qT = qfp.tile([P, KT, P], f8)
            nc.vector.tensor_copy(qT[:], qTs[:])

            # GEMMs: (32 int + 2 outlier) matmuls per psum chunk
            psum = py.tile([P, os_dim], f32)
            if mi < 4:
                # ramp tiles: N=128 per weight 128-chunk, emitted chunk-major
                # so each chunk's matmuls start as soon as ITS unpack lands
                # (N=512 needs 4 chunks; j-interleaved needs all 8)
                for c in range(OC):
                    j, cc = c // (OC // NJ), c % (OC // NJ)
                    pslice = psum[:, c * P : (c + 1) * P]
                    for ko in range(KT):
                        nc.tensor.matmul(
                            pslice,
                            qT[:, ko, :],
                            wT[j][:, ko, cc * P : (cc + 1) * P],
                            start=(ko == 0),
                            stop=False,
                        )
                    for ff in range(FT):
                        nc.tensor.matmul(
                            pslice,
                            aoT[:, ff, :],
                            wcT[:, ff, c * P : (c + 1) * P],
                            start=False,
                            stop=(ff == FT - 1),
                        )
            else:
                for t in range(KT // 2):
                    for j in range(NJ):
                        nc.tensor.matmul(
                            psum[:, j * 512 : (j + 1) * 512],
                            qT[:, 2 * t : 2 * t + 2, :],
                            wT[j][:, 2 * t : 2 * t + 2, :],
                            perf_mode=mybir.MatmulPerfMode.DoubleRow,
                            start=(t == 0),
                            stop=False,
                        )
                for ff in range(FT):
                    for j in range(NJ):
                        nc.tensor.matmul(
                            psum[:, j * 512 : (j + 1) * 512],
                            aoT[:, ff, :],
                            wcT[:, ff, j * 512 : (j + 1) * 512],
                            start=False,
                            stop=(ff == FT - 1),
                        )

            # dequant + store: y = (psum * x_scale) * scale_col
            yt = yp.tile([P, os_dim], f32, tag="yt")
            if "evict" in SAFE:
                t1 = yp.tile([P, os_dim], f32, tag="t1")
                nc.scalar.activation(t1[:], psum[:], Act.Copy, scale=s_t[:])
                nc.vector.scalar_tensor_tensor(
                    yt[:], t1[:], 1.0, sc_bcast[:], Alu.mult, Alu.mult
                )
            else:
                nc.vector.scalar_tensor_tensor(
                    yt[:], psum[:], s_t[:], sc_bcast[:], Alu.mult, Alu.mult
                )
            nc.sync.dma_start(y[mi * P : (mi + 1) * P, :], yt[:])

    return nc


def build_nc(m=MS, in_dim=IN, os_dim=OS, fp_dim=FP):
    import concourse.bacc as bacc
    import concourse.tile as tile

    nc = bacc.Bacc(None, target_bir_lowering=False)
    with tile.TileContext(nc) as tc:
        emit_core_kernel(nc, tc, m, in_dim, os_dim, fp_dim)
    nc.compile()
    return nc


def make_host_inputs(x, q_weight, scale_col, weight_cache, ind,
                     in_dim=IN, os_dim=OS, ms=MS, fp_dim=FP):
    """Shard/relayout full inputs into per-core input maps (no arithmetic)."""
    xf = np.ascontiguousarray(x.reshape(M, in_dim).astype(np.float32, copy=False))
    ind = np.asarray(ind).astype(np.int64)
    maskrow = np.ones(in_dim, dtype=np.float32)
    maskrow[ind] = 0.0
    w = ind.astype(np.int16).reshape(fp_dim // 16, 16)  # j = i*16 + (p%16)
    idx = np.tile(w.T, (8, 1)).astype(np.int16)  # [128, fp/16]
    scf = np.asarray(scale_col).reshape(-1).astype(np.float32, copy=False)

    in_maps = []
    for c in range(NCORES):
        mg, og = c // OG, c % OG
        o0, o1 = og * os_dim, (og + 1) * os_dim
        in_maps.append(
            {
                "x": xf[mg * ms : (mg + 1) * ms],
                "qw": np.ascontiguousarray(q_weight[o0:o1]).astype(np.int32, copy=False),
                "sc": np.ascontiguousarray(scf[o0:o1]),
                "wc": np.ascontiguousarray(weight_cache[o0:o1]).astype(np.float32, copy=False),
                "maskrow": maskrow,
                "idx": idx,
            }
        )
    return in_maps


_NC_CACHE = {}


def kernel(x, q_weight, scale_col, weight_cache, ind, trace=False):
    from concourse.bass_utils import run_bass_kernel_spmd

    key = "full"
    if key not in _NC_CACHE:
        _NC_CACHE[key] = build_nc()
    nc = _NC_CACHE[key]

    in_maps = make_host_inputs(x, q_weight, scale_col, weight_cache, ind)
    res = run_bass_kernel_spmd(nc, in_maps, list(range(NCORES)), trace=trace)
    yfull = np.empty((M, OUT), dtype=np.float32)
    for c in range(NCORES):
        mg, og = c // OG, c % OG
        yfull[mg * MS : (mg + 1) * MS, og * OS : (og + 1) * OS] = res.results[c]["y"]
    yfull = yfull.reshape(B, S, OUT)
    if trace:
        return yfull, res
    return yfull


# revision 33
# speedup vs baseline: 1.0862x; 1.0862x over previous
"""MixLinear int4-GEMM kernel for 8x TRN2 NeuronCores.

Strategy: 2-way M x 4-way OUT sharding.  Core c = mg*4 + og owns rows
[mg*4096, (mg+1)*4096) and output channels [og*1024, (og+1)*1024).  This
splits the per-row quantization work 4x vs pure OUT-sharding (which
duplicated it on all 8 cores) while keeping the whole bf16 weight shard
resident in SBUF.

Per core:
  Setup (once): int4 weight shard unpacked on DVE into bf16 and DMA-xbar
  transposed to contraction-major wT; outlier weight columns wc/sc
  (pre-divided by scale_col so one dequant covers everything) transposed
  via PE into wcT bf16.
  Per 128-row tile (32 tiles): masked abs-max -> x_scale; magic-number
  RNE round on ScalarE+DVE -> q (exact ints in bf16); DMA-xbar transpose
  to qT; GPSIMD outlier gather + ScalarE scale + PE transpose; 32+2 bf16
  matmuls into a [128, 1024] PSUM pair; dequant eviction; DMA out.

KERNEL_SAFE env (comma list) falls back to baseline-proven constructs:
  mask  - f32 mask, masked quantize (TT+reduce, no fused TTR / weight-col
          zeroing / in-place DVE)
  evict - two-step dequant (ScalarE scale, then DVE col-scale)
  qtp1  - single-buffered qT
"""

import os as _os

import numpy as np

B, S, IN, OUT, FP = 4, 2048, 4096, 4096, 256
M = B * S
NCORES = 8
MG, OG = 2, 4            # M-groups x OUT-groups
MS = M // MG             # rows per core (4096)
OS = OUT // OG           # out-features per core (1024)
QMAX = 7.0
MAGIC = 12582912.0       # 1.5 * 2**23: add+subtract forces RNE to integer


def emit_core_kernel(nc, tc, m, in_dim, os_dim, fp_dim):
    """Emit the per-core tile program. All dims compile-time constants."""
    import concourse.mybir as mybir
    from concourse import library_config
    from concourse.masks import make_identity

    SAFE = set(_os.environ.get("KERNEL_SAFE", "").split(","))

    f32 = mybir.dt.float32
    f8 = mybir.dt.float8e4
    bf16 = mybir.dt.bfloat16
    i32 = mybir.dt.int32
    i16 = mybir.dt.int16
    Alu = mybir.AluOpType
    Act = mybir.ActivationFunctionType

    P = 128
    MT = m // P              # 32 activation row tiles
    KT = in_dim // P         # 32 contraction tiles
    FT = fp_dim // P         # 2 outlier contraction tiles
    OC = os_dim // P         # 8 out-shard 128-chunks
    NJ = os_dim // 512       # 2 psum chunks of 512

    x = nc.dram_tensor("x", [m, in_dim], f32, kind="ExternalInput")
    qw = nc.dram_tensor("qw", [os_dim, in_dim // 2], i32, kind="ExternalInput")
    sc = nc.dram_tensor("sc", [os_dim], f32, kind="ExternalInput")
    wc = nc.dram_tensor("wc", [os_dim, fp_dim], f32, kind="ExternalInput")
    maskrow = nc.dram_tensor("maskrow", [in_dim], f32, kind="ExternalInput")
    idx = nc.dram_tensor("idx", [P, fp_dim // 16], i16, kind="ExternalInput")
    y = nc.dram_tensor("y", [m, os_dim], f32, kind="ExternalOutput")

    with (
        tc.tile_pool(name="const", bufs=1) as const,
        tc.tile_pool(name="wstage", bufs=1) as wstage,
        tc.tile_pool(name="xp", bufs=2) as xp,
        tc.tile_pool(name="xzp", bufs=1) as xzp,
        tc.tile_pool(name="qp", bufs=2) as qp,
        tc.tile_pool(name="qtp", bufs=1 if "qtp1" in SAFE else 2) as qtp,
        tc.tile_pool(name="qfp", bufs=2) as qfp,
        tc.tile_pool(name="aop", bufs=2) as aop,
        tc.tile_pool(name="sp", bufs=8) as sp,
        tc.tile_pool(name="yp", bufs=1 if "evict" in SAFE else 2) as yp,
        tc.tile_pool(name="py", bufs=3, space="PSUM") as py,
        tc.tile_pool(name="ptp", bufs=2, space="PSUM") as ptp,
    ):
        # ---------------- one-time setup ----------------
        nc.gpsimd.load_library(library_config.ap_gather)

        identity = const.tile([P, P], f32)
        make_identity(nc, identity[:])

        # outlier mask broadcast to all partitions, bf16 (0/1 exact;
        # halves the mask read bandwidth in the masking TT pass)
        maskF = const.tile([P, in_dim], bf16, name="maskF")
        mtmp = xzp.tile([P, in_dim], f32, tag="xz")
        nc.sync.dma_start(mtmp[:], maskrow[None, :].to_broadcast((P, in_dim)))
        nc.scalar.activation(maskF[:], mtmp[:], Act.Copy)

        # wrapped gather indices for ap_gather
        idxs = const.tile([P, fp_dim // 16], i16)
        nc.sync.dma_start(idxs[:], idx[:])

        # scale_col shard broadcast along partitions [P, OS] for dequant
        sc_bcast = const.tile([P, os_dim], f32)
        nc.sync.dma_start(sc_bcast[:], sc[None, :].to_broadcast((P, os_dim)))

        # scale_col per-partition view [P, OC] for pre-dividing weight_cache
        sc_op = const.tile([P, OC], f32)
        nc.sync.dma_start(sc_op[:], sc.rearrange("(c p) -> p c", p=P))
        rsc_op = const.tile([P, OC], f32)
        nc.vector.reciprocal(rsc_op[:], sc_op[:])

        # int4 weight unpack into contraction-major bf16 wT (per j-half so
        # early matmuls only wait on their own half), and weight_cache
        # pre-scaled by 1/scale_col into wcT.
        wT = [
            const.tile([P, KT, 512], f8, name=f"wT{j}", tag=f"wT{j}")
            for j in range(NJ)
        ]
        wcT = const.tile([P, FT, os_dim], bf16)
        qw_v = qw.rearrange("(c p) k -> p c k", p=P)
        wc_v = wc.rearrange("(c p) f -> p c f", p=P)
        for c in range(OC):
            qwj = wstage.tile([P, in_dim // 2], i32, tag="qwj", bufs=2)
            nc.sync.dma_start(qwj[:], qw_v[:, c, :])
            w_ok = wstage.tile([P, in_dim], bf16, tag="wok")
            w_ok_v = w_ok.rearrange("p (k two) -> p k two", two=2)
            tmp = wstage.tile([P, in_dim // 2], i32, tag="wtmp")
            # low nibble: ((v & 15) ^ 8) - 8
            nc.vector.tensor_scalar(
                tmp[:], qwj[:], 15, 8, Alu.bitwise_and, Alu.bitwise_xor
            )
            nc.vector.tensor_scalar(w_ok_v[:, :, 0], tmp[:], 8, None, Alu.subtract)
            # high nibble: same decode after v >>= 4 (ping-pong, no in-place)
            nc.vector.tensor_scalar(tmp[:], qwj[:], 4, None, Alu.arith_shift_right)
            nc.vector.tensor_scalar(
                qwj[:], tmp[:], 15, 8, Alu.bitwise_and, Alu.bitwise_xor
            )
            nc.vector.tensor_scalar(w_ok_v[:, :, 1], qwj[:], 8, None, Alu.subtract)
            # transpose [128 o, in_dim k] -> staging, then convert to fp8
            # (exact: int4 values; DMA transpose only supports 2B dtypes)
            j, cc = c // (OC // NJ), c % (OC // NJ)
            wTs = wstage.tile([P, KT, P], bf16, tag="wts", bufs=2)
            nc.sync.dma_start_transpose(wTs[:], w_ok[:])
            nc.scalar.activation(
                wT[j][:, :, cc * P : (cc + 1) * P], wTs[:], Act.Copy
            )

            # outlier weights: wcp = wc[o, f] / sc[o], PE transpose (f32
            # PSUM; converted to bf16 at the ScalarE eviction)
            wcc = wstage.tile([P, fp_dim], f32, tag="wcc")
            nc.sync.dma_start(wcc[:], wc_v[:, c, :])
            wcp = wstage.tile([P, fp_dim], f32, tag="wcp")
            nc.scalar.activation(
                wcp[:], wcc[:], Act.Copy, scale=rsc_op[:, c : c + 1]
            )
            for ff in range(FT):
                ps = ptp.tile([P, P], f32, tag="tp")
                nc.tensor.transpose(
                    ps[:], wcp[:, ff * P : (ff + 1) * P], identity[:]
                )
                nc.scalar.activation(
                    wcT[:, ff, c * P : (c + 1) * P], ps[:], Act.Copy
                )

        # ---------------- main loop over 128-row activation tiles ----------
        inv7 = float(np.float32(1.0) / np.float32(QMAX))
        for mi in range(MT):
            x_t = xp.tile([P, in_dim], f32)
            nc.sync.dma_start(x_t[:], x[mi * P : (mi + 1) * P, :])

            # outlier activations (full precision, pre-masking)
            ao = aop.tile([P, fp_dim], f32, tag="ao")
            nc.gpsimd.ap_gather(
                ao[:, :, None],
                x_t[:, :, None],
                idxs[:],
                channels=P,
                num_elems=in_dim,
                d=1,
                num_idxs=fp_dim,
            )

            # xz = x*mask (one DVE pass, bf16 mask); mx = absmax(xz).
            # NOTE: the fused tensor_tensor_reduce hangs real TRN2 hardware
            # (passes CoreSim) -- keep the two-instruction form.
            axs = xzp.tile([P, in_dim], f32, tag="xz")
            mx = sp.tile([P, 1], f32, tag="mx")
            nc.vector.tensor_tensor(axs[:], x_t[:], maskF[:], Alu.mult)
            nc.vector.tensor_reduce(
                mx[:], axs[:], mybir.AxisListType.X, Alu.max,
                apply_absolute_value=True,
            )
            qsrc = axs
            s_t = sp.tile([P, 1], f32, tag="s")
            nc.vector.tensor_scalar(s_t[:], mx[:], inv7, None, Alu.mult)
            r_t = sp.tile([P, 1], f32, tag="r")
            nc.vector.reciprocal(r_t[:], s_t[:])

            # outliers scaled by r (ScalarE; per-partition scale), PE transpose
            aos = aop.tile([P, fp_dim], f32, tag="aos")
            nc.scalar.activation(aos[:], ao[:], Act.Copy, scale=r_t[:])
            aoT = aop.tile([P, FT, P], bf16, tag="aoT", bufs=1)
            for ff in range(FT):
                ps = ptp.tile([P, P], f32, tag="tp")
                nc.tensor.transpose(
                    ps[:], aos[:, ff * P : (ff + 1) * P], identity[:]
                )
                nc.scalar.activation(aoT[:, ff, :], ps[:], Act.Copy)

            # quantize: t = qsrc * r + MAGIC (ScalarE), q = t - MAGIC (DVE 2x).
            # t lands in whichever of x_t/axs is no longer needed.
            tq = x_t if qsrc is axs else axs
            nc.scalar.activation(tq[:], qsrc[:], Act.Copy, bias=MAGIC, scale=r_t[:])
            q = qp.tile([P, in_dim], bf16, tag="q")
            # magic-subtract split across DVE and ScalarE to balance load
            h = in_dim // 2
            nc.vector.tensor_scalar(q[:, :h], tq[:, :h], -MAGIC, None, Alu.add)
            nc.scalar.activation(q[:, h:], tq[:, h:], Act.Copy, bias=-MAGIC)

            # transpose q to contraction-major via DMA xbar (bf16), then
            # convert to fp8 for the DoubleRow matmuls (exact ints;
            # ScalarE -- fp8 writes are unpacked 1x on the DVE)
            qTs = qtp.tile([P, KT, P], bf16)
            nc.sync.dma_start_transpose(qTs[:], q[:])
            qT = qfp.tile([P, KT, P], f8)
            nc.scalar.activation(qT[:], qTs[:], Act.Copy)

            # GEMMs: (32 int + 2 outlier) matmuls per psum chunk
            psum = py.tile([P, os_dim], f32)
            if mi < 4:
                # ramp tiles: N=128 per weight 128-chunk, emitted chunk-major
                # so each chunk's matmuls start as soon as ITS unpack lands
                # (N=512 needs 4 chunks; j-interleaved needs all 8)
                for c in range(OC):
                    j, cc = c // (OC // NJ), c % (OC // NJ)
                    pslice = psum[:, c * P : (c + 1) * P]
                    for ko in range(KT):
                        nc.tensor.matmul(
                            pslice,
                            qT[:, ko, :],
                            wT[j][:, ko, cc * P : (cc + 1) * P],
                            start=(ko == 0),
                            stop=False,
                        )
                    for ff in range(FT):
                        nc.tensor.matmul(
                            pslice,
                            aoT[:, ff, :],
                            wcT[:, ff, c * P : (c + 1) * P],
                            start=False,
                            stop=(ff == FT - 1),
                        )
            else:
                for t in range(KT // 2):
                    for j in range(NJ):
                        nc.tensor.matmul(
                            psum[:, j * 512 : (j + 1) * 512],
                            qT[:, 2 * t : 2 * t + 2, :],
                            wT[j][:, 2 * t : 2 * t + 2, :],
                            perf_mode=mybir.MatmulPerfMode.DoubleRow,
                            start=(t == 0),
                            stop=False,
                        )
                for ff in range(FT):
                    for j in range(NJ):
                        nc.tensor.matmul(
                            psum[:, j * 512 : (j + 1) * 512],
                            aoT[:, ff, :],
                            wcT[:, ff, j * 512 : (j + 1) * 512],
                            start=False,
                            stop=(ff == FT - 1),
                        )

            # dequant + store: y = (psum * x_scale) * scale_col
            yt = yp.tile([P, os_dim], f32, tag="yt")
            if "evict" in SAFE:
                t1 = yp.tile([P, os_dim], f32, tag="t1")
                nc.scalar.activation(t1[:], psum[:], Act.Copy, scale=s_t[:])
                nc.vector.scalar_tensor_tensor(
                    yt[:], t1[:], 1.0, sc_bcast[:], Alu.mult, Alu.mult
                )
            else:
                nc.vector.scalar_tensor_tensor(
                    yt[:], psum[:], s_t[:], sc_bcast[:], Alu.mult, Alu.mult
                )
            nc.sync.dma_start(y[mi * P : (mi + 1) * P, :], yt[:])

    return nc


def build_nc(m=MS, in_dim=IN, os_dim=OS, fp_dim=FP):
    import concourse.bacc as bacc
    import concourse.tile as tile

    nc = bacc.Bacc(None, target_bir_lowering=False)
    with tile.TileContext(nc) as tc:
        emit_core_kernel(nc, tc, m, in_dim, os_dim, fp_dim)
    nc.compile()
    return nc


def make_host_inputs(x, q_weight, scale_col, weight_cache, ind,
                     in_dim=IN, os_dim=OS, ms=MS, fp_dim=FP):
    """Shard/relayout full inputs into per-core input maps (no arithmetic)."""
    xf = np.ascontiguousarray(x.reshape(M, in_dim).astype(np.float32, copy=False))
    ind = np.asarray(ind).astype(np.int64)
    maskrow = np.ones(in_dim, dtype=np.float32)
    maskrow[ind] = 0.0
    w = ind.astype(np.int16).reshape(fp_dim // 16, 16)  # j = i*16 + (p%16)
    idx = np.tile(w.T, (8, 1)).astype(np.int16)  # [128, fp/16]
    scf = np.asarray(scale_col).reshape(-1).astype(np.float32, copy=False)

    in_maps = []
    for c in range(NCORES):
        mg, og = c // OG, c % OG
        o0, o1 = og * os_dim, (og + 1) * os_dim
        in_maps.append(
            {
                "x": xf[mg * ms : (mg + 1) * ms],
                "qw": np.ascontiguousarray(q_weight[o0:o1]).astype(np.int32, copy=False),
                "sc": np.ascontiguousarray(scf[o0:o1]),
                "wc": np.ascontiguousarray(weight_cache[o0:o1]).astype(np.float32, copy=False),
                "maskrow": maskrow,
                "idx": idx,
            }
        )
    return in_maps


_NC_CACHE = {}


def kernel(x, q_weight, scale_col, weight_cache, ind, trace=False):
    from concourse.bass_utils import run_bass_kernel_spmd

    key = "full"
    if key not in _NC_CACHE:
        _NC_CACHE[key] = build_nc()
    nc = _NC_CACHE[key]

    in_maps = make_host_inputs(x, q_weight, scale_col, weight_cache, ind)
    res = run_bass_kernel_spmd(nc, in_maps, list(range(NCORES)), trace=trace)
    yfull = np.empty((M, OUT), dtype=np.float32)
    for c in range(NCORES):
        mg, og = c // OG, c % OG
        yfull[mg * MS : (mg + 1) * MS, og * OS : (og + 1) * OS] = res.results[c]["y"]
    yfull = yfull.reshape(B, S, OUT)
    if trace:
        return yfull, res
    return yfull


# revision 39
# speedup vs baseline: 1.0862x; 1.0000x over previous
"""MixLinear int4-GEMM kernel for 8x TRN2 NeuronCores.

Strategy: 2-way M x 4-way OUT sharding.  Core c = mg*4 + og owns rows
[mg*4096, (mg+1)*4096) and output channels [og*1024, (og+1)*1024).  This
splits the per-row quantization work 4x vs pure OUT-sharding (which
duplicated it on all 8 cores) while keeping the whole bf16 weight shard
resident in SBUF.

Per core:
  Setup (once): int4 weight shard unpacked on DVE into bf16 and DMA-xbar
  transposed to contraction-major wT; outlier weight columns wc/sc
  (pre-divided by scale_col so one dequant covers everything) transposed
  via PE into wcT bf16.
  Per 128-row tile (32 tiles): masked abs-max -> x_scale; magic-number
  RNE round on ScalarE+DVE -> q (exact ints in bf16); DMA-xbar transpose
  to qT; GPSIMD outlier gather + ScalarE scale + PE transpose; 32+2 bf16
  matmuls into a [128, 1024] PSUM pair; dequant eviction; DMA out.

KERNEL_SAFE env (comma list) falls back to baseline-proven constructs:
  mask  - f32 mask, masked quantize (TT+reduce, no fused TTR / weight-col
          zeroing / in-place DVE)
  evict - two-step dequant (ScalarE scale, then DVE col-scale)
  qtp1  - single-buffered qT
"""

import os as _os

import numpy as np

B, S, IN, OUT, FP = 4, 2048, 4096, 4096, 256
M = B * S
NCORES = 8
MG, OG = 2, 4            # M-groups x OUT-groups
MS = M // MG             # rows per core (4096)
OS = OUT // OG           # out-features per core (1024)
QMAX = 7.0
MAGIC = 12582912.0       # 1.5 * 2**23: add+subtract forces RNE to integer


def emit_core_kernel(nc, tc, m, in_dim, os_dim, fp_dim):
    """Emit the per-core tile program. All dims compile-time constants."""
    import concourse.mybir as mybir
    from concourse import library_config
    from concourse.masks import make_identity

    SAFE = set(_os.environ.get("KERNEL_SAFE", "").split(","))

    f32 = mybir.dt.float32
    f8 = mybir.dt.float8e4
    bf16 = mybir.dt.bfloat16
    i32 = mybir.dt.int32
    i16 = mybir.dt.int16
    Alu = mybir.AluOpType
    Act = mybir.ActivationFunctionType

    P = 128
    MT = m // P              # 32 activation row tiles
    KT = in_dim // P         # 32 contraction tiles
    FT = fp_dim // P         # 2 outlier contraction tiles
    OC = os_dim // P         # 8 out-shard 128-chunks
    NJ = os_dim // 512       # 2 psum chunks of 512

    x = nc.dram_tensor("x", [m, in_dim], f32, kind="ExternalInput")
    qw = nc.dram_tensor("qw", [os_dim, in_dim // 2], i32, kind="ExternalInput")
    sc = nc.dram_tensor("sc", [os_dim], f32, kind="ExternalInput")
    wc = nc.dram_tensor("wc", [os_dim, fp_dim], f32, kind="ExternalInput")
    maskrow = nc.dram_tensor("maskrow", [in_dim], f32, kind="ExternalInput")
    idx = nc.dram_tensor("idx", [P, fp_dim // 16], i16, kind="ExternalInput")
    y = nc.dram_tensor("y", [m, os_dim], f32, kind="ExternalOutput")

    with (
        tc.tile_pool(name="const", bufs=1) as const,
        tc.tile_pool(name="wstage", bufs=1) as wstage,
        tc.tile_pool(name="xp", bufs=2) as xp,
        tc.tile_pool(name="xzp", bufs=1) as xzp,
        tc.tile_pool(name="qp", bufs=2) as qp,
        tc.tile_pool(name="qtp", bufs=1 if "qtp1" in SAFE else 2) as qtp,
        tc.tile_pool(name="qfp", bufs=2) as qfp,
        tc.tile_pool(name="aop", bufs=2) as aop,
        tc.tile_pool(name="sp", bufs=8) as sp,
        tc.tile_pool(name="yp", bufs=1 if "evict" in SAFE else 2) as yp,
        tc.tile_pool(name="py", bufs=3, space="PSUM") as py,
        tc.tile_pool(name="ptp", bufs=2, space="PSUM") as ptp,
    ):
        # ---------------- one-time setup ----------------
        nc.gpsimd.load_library(library_config.ap_gather)

        identity = const.tile([P, P], f32)
        make_identity(nc, identity[:])

        # outlier mask broadcast to all partitions, bf16 (0/1 exact;
        # halves the mask read bandwidth in the masking TT pass)
        maskF = const.tile([P, in_dim], bf16, name="maskF")
        mtmp = xzp.tile([P, in_dim], f32, tag="xz")
        nc.sync.dma_start(mtmp[:], maskrow[None, :].to_broadcast((P, in_dim)))
        nc.scalar.activation(maskF[:], mtmp[:], Act.Copy)

        # wrapped gather indices for ap_gather
        idxs = const.tile([P, fp_dim // 16], i16)
        nc.sync.dma_start(idxs[:], idx[:])

        # scale_col shard broadcast along partitions [P, OS] for dequant
        sc_bcast = const.tile([P, os_dim], f32)
        nc.sync.dma_start(sc_bcast[:], sc[None, :].to_broadcast((P, os_dim)))

        # scale_col per-partition view [P, OC] for pre-dividing weight_cache
        sc_op = const.tile([P, OC], f32)
        nc.sync.dma_start(sc_op[:], sc.rearrange("(c p) -> p c", p=P))
        rsc_op = const.tile([P, OC], f32)
        nc.vector.reciprocal(rsc_op[:], sc_op[:])

        # int4 weight unpack into contraction-major bf16 wT (per j-half so
        # early matmuls only wait on their own half), and weight_cache
        # pre-scaled by 1/scale_col into wcT.
        wT = [
            const.tile([P, KT, 512], f8, name=f"wT{j}", tag=f"wT{j}")
            for j in range(NJ)
        ]
        wcT = const.tile([P, FT, os_dim], bf16)
        qw_v = qw.rearrange("(c p) k -> p c k", p=P)
        wc_v = wc.rearrange("(c p) f -> p c f", p=P)
        for c in range(OC):
            qwj = wstage.tile([P, in_dim // 2], i32, tag="qwj", bufs=2)
            nc.sync.dma_start(qwj[:], qw_v[:, c, :])
            w_ok = wstage.tile([P, in_dim], bf16, tag="wok")
            w_ok_v = w_ok.rearrange("p (k two) -> p k two", two=2)
            tmp = wstage.tile([P, in_dim // 2], i32, tag="wtmp")
            # low nibble: ((v & 15) ^ 8) - 8
            nc.vector.tensor_scalar(
                tmp[:], qwj[:], 15, 8, Alu.bitwise_and, Alu.bitwise_xor
            )
            nc.vector.tensor_scalar(w_ok_v[:, :, 0], tmp[:], 8, None, Alu.subtract)
            # high nibble: same decode after v >>= 4 (ping-pong, no in-place)
            nc.vector.tensor_scalar(tmp[:], qwj[:], 4, None, Alu.arith_shift_right)
            nc.vector.tensor_scalar(
                qwj[:], tmp[:], 15, 8, Alu.bitwise_and, Alu.bitwise_xor
            )
            nc.vector.tensor_scalar(w_ok_v[:, :, 1], qwj[:], 8, None, Alu.subtract)
            # transpose [128 o, in_dim k] -> staging, then convert to fp8
            # (exact: int4 values; DMA transpose only supports 2B dtypes)
            j, cc = c // (OC // NJ), c % (OC // NJ)
            wTs = wstage.tile([P, KT, P], bf16, tag="wts", bufs=2)
            nc.sync.dma_start_transpose(wTs[:], w_ok[:])
            nc.scalar.activation(
                wT[j][:, :, cc * P : (cc + 1) * P], wTs[:], Act.Copy
            )

            # outlier weights: wcp = wc[o, f] / sc[o], PE transpose (f32
            # PSUM; converted to bf16 at the ScalarE eviction)
            wcc = wstage.tile([P, fp_dim], f32, tag="wcc")
            nc.sync.dma_start(wcc[:], wc_v[:, c, :])
            wcp = wstage.tile([P, fp_dim], f32, tag="wcp")
            nc.scalar.activation(
                wcp[:], wcc[:], Act.Copy, scale=rsc_op[:, c : c + 1]
            )
            for ff in range(FT):
                ps = ptp.tile([P, P], f32, tag="tp")
                nc.tensor.transpose(
                    ps[:], wcp[:, ff * P : (ff + 1) * P], identity[:]
                )
                nc.scalar.activation(
                    wcT[:, ff, c * P : (c + 1) * P], ps[:], Act.Copy
                )

        # ---------------- main loop over 128-row activation tiles ----------
        inv7 = float(np.float32(1.0) / np.float32(QMAX))
        for mi in range(MT):
            x_t = xp.tile([P, in_dim], f32)
            nc.sync.dma_start(x_t[:], x[mi * P : (mi + 1) * P, :])

            # outlier activations (full precision, pre-masking)
            ao = aop.tile([P, fp_dim], f32, tag="ao")
            nc.gpsimd.ap_gather(
                ao[:, :, None],
                x_t[:, :, None],
                idxs[:],
                channels=P,
                num_elems=in_dim,
                d=1,
                num_idxs=fp_dim,
            )

            # xz = x*mask (one DVE pass, bf16 mask); mx = absmax(xz).
            # NOTE: the fused tensor_tensor_reduce hangs real TRN2 hardware
            # (passes CoreSim) -- keep the two-instruction form.
            axs = xzp.tile([P, in_dim], f32, tag="xz")
            mx = sp.tile([P, 1], f32, tag="mx")
            nc.vector.tensor_tensor(axs[:], x_t[:], maskF[:], Alu.mult)
            nc.vector.tensor_reduce(
                mx[:], axs[:], mybir.AxisListType.X, Alu.max,
                apply_absolute_value=True,
            )
            qsrc = axs
            s_t = sp.tile([P, 1], f32, tag="s")
            nc.vector.tensor_scalar(s_t[:], mx[:], inv7, None, Alu.mult)
            r_t = sp.tile([P, 1], f32, tag="r")
            nc.vector.reciprocal(r_t[:], s_t[:])

            # outliers scaled by r (ScalarE; per-partition scale), PE transpose
            aos = aop.tile([P, fp_dim], f32, tag="aos")
            nc.scalar.activation(aos[:], ao[:], Act.Copy, scale=r_t[:])
            aoT = aop.tile([P, FT, P], bf16, tag="aoT", bufs=1)
            for ff in range(FT):
                ps = ptp.tile([P, P], f32, tag="tp")
                nc.tensor.transpose(
                    ps[:], aos[:, ff * P : (ff + 1) * P], identity[:]
                )
                nc.scalar.activation(aoT[:, ff, :], ps[:], Act.Copy)

            # quantize: t = qsrc * r + MAGIC (ScalarE), q = t - MAGIC (DVE 2x).
            # t lands in whichever of x_t/axs is no longer needed.
            tq = x_t if qsrc is axs else axs
            nc.scalar.activation(tq[:], qsrc[:], Act.Copy, bias=MAGIC, scale=r_t[:])
            q = qp.tile([P, in_dim], bf16, tag="q")
            # magic-subtract split across DVE and ScalarE to balance load
            h = in_dim // 2
            nc.vector.tensor_scalar(q[:, :h], tq[:, :h], -MAGIC, None, Alu.add)
            nc.scalar.activation(q[:, h:], tq[:, h:], Act.Copy, bias=-MAGIC)

            # transpose q to contraction-major via DMA xbar (bf16), then
            # convert to fp8 for the DoubleRow matmuls (exact ints;
            # ScalarE -- fp8 writes are unpacked 1x on the DVE)
            qTs = qtp.tile([P, KT, P], bf16)
            nc.sync.dma_start_transpose(qTs[:], q[:])
            qT = qfp.tile([P, KT, P], f8)
            nc.scalar.activation(qT[:], qTs[:], Act.Copy)

            # GEMMs: (32 int + 2 outlier) matmuls per psum chunk
            psum = py.tile([P, os_dim], f32)
            if mi < 4:
                # ramp tiles: N=128 per weight 128-chunk, emitted chunk-major
                # so each chunk's matmuls start as soon as ITS unpack lands
                # (N=512 needs 4 chunks; j-interleaved needs all 8)
                for c in range(OC):
                    j, cc = c // (OC // NJ), c % (OC // NJ)
                    pslice = psum[:, c * P : (c + 1) * P]
                    for ko in range(KT):
                        nc.tensor.matmul(
                            pslice,
                            qT[:, ko, :],
                            wT[j][:, ko, cc * P : (cc + 1) * P],
                            start=(ko == 0),
                            stop=False,
                        )
                    for ff in range(FT):
                        nc.tensor.matmul(
                            pslice,
                            aoT[:, ff, :],
                            wcT[:, ff, c * P : (c + 1) * P],
                            start=False,
                            stop=(ff == FT - 1),
                        )
            else:
                for t in range(KT // 2):
                    for j in range(NJ):
                        nc.tensor.matmul(
                            psum[:, j * 512 : (j + 1) * 512],
                            qT[:, 2 * t : 2 * t + 2, :],
                            wT[j][:, 2 * t : 2 * t + 2, :],
                            perf_mode=mybir.MatmulPerfMode.DoubleRow,
                            start=(t == 0),
                            stop=False,
                        )
                for ff in range(FT):
                    for j in range(NJ):
                        nc.tensor.matmul(
                            psum[:, j * 512 : (j + 1) * 512],
                            aoT[:, ff, :],
                            wcT[:, ff, j * 512 : (j + 1) * 512],
                            start=False,
                            stop=(ff == FT - 1),
                        )

            # dequant + store: y = (psum * x_scale) * scale_col
            yt = yp.tile([P, os_dim], f32, tag="yt")
            if "evict" in SAFE:
                t1 = yp.tile([P, os_dim], f32, tag="t1")
                nc.scalar.activation(t1[:], psum[:], Act.Copy, scale=s_t[:])
                nc.vector.scalar_tensor_tensor(
                    yt[:], t1[:], 1.0, sc_bcast[:], Alu.mult, Alu.mult
                )
            else:
                nc.vector.scalar_tensor_tensor(
                    yt[:], psum[:], s_t[:], sc_bcast[:], Alu.mult, Alu.mult
                )
            nc.sync.dma_start(y[mi * P : (mi + 1) * P, :], yt[:])

    return nc


def build_nc(m=MS, in_dim=IN, os_dim=OS, fp_dim=FP):
    import concourse.bacc as bacc
    import concourse.tile as tile

    nc = bacc.Bacc(None, target_bir_lowering=False)
    with tile.TileContext(nc) as tc:
        emit_core_kernel(nc, tc, m, in_dim, os_dim, fp_dim)
    nc.compile()
    return nc


def make_host_inputs(x, q_weight, scale_col, weight_cache, ind,
                     in_dim=IN, os_dim=OS, ms=MS, fp_dim=FP):
    """Shard/relayout full inputs into per-core input maps (no arithmetic)."""
    xf = np.ascontiguousarray(x.reshape(M, in_dim).astype(np.float32, copy=False))
    ind = np.asarray(ind).astype(np.int64)
    maskrow = np.ones(in_dim, dtype=np.float32)
    maskrow[ind] = 0.0
    w = ind.astype(np.int16).reshape(fp_dim // 16, 16)  # j = i*16 + (p%16)
    idx = np.tile(w.T, (8, 1)).astype(np.int16)  # [128, fp/16]
    scf = np.asarray(scale_col).reshape(-1).astype(np.float32, copy=False)

    in_maps = []
    for c in range(NCORES):
        mg, og = c // OG, c % OG
        o0, o1 = og * os_dim, (og + 1) * os_dim
        in_maps.append(
            {
                "x": xf[mg * ms : (mg + 1) * ms],
                "qw": np.ascontiguousarray(q_weight[o0:o1]).astype(np.int32, copy=False),
                "sc": np.ascontiguousarray(scf[o0:o1]),
                "wc": np.ascontiguousarray(weight_cache[o0:o1]).astype(np.float32, copy=False),
                "maskrow": maskrow,
                "idx": idx,
            }
        )
    return in_maps


_NC_CACHE = {}


def kernel(x, q_weight, scale_col, weight_cache, ind, trace=False):
    from concourse.bass_utils import run_bass_kernel_spmd

    key = "full"
    if key not in _NC_CACHE:
        _NC_CACHE[key] = build_nc()
    nc = _NC_CACHE[key]

    in_maps = make_host_inputs(x, q_weight, scale_col, weight_cache, ind)
    res = run_bass_kernel_spmd(nc, in_maps, list(range(NCORES)), trace=trace)
    yfull = np.empty((M, OUT), dtype=np.float32)
    for c in range(NCORES):
        mg, og = c // OG, c % OG
        yfull[mg * MS : (mg + 1) * MS, og * OS : (og + 1) * OS] = res.results[c]["y"]
    yfull = yfull.reshape(B, S, OUT)
    if trace:
        return yfull, res
    return yfull
